# revision 21
# baseline (speedup 1.0000x reference)
"""Trainium2 Bass kernel for 16-head causal self-attention (KaplanAttention).

Problem: x [2, 2048, 1024], torch-style weights W_q/W_k/W_v/W_o [1024, 1024].
  q/k/v = (x @ W.T) split into 16 heads of 64; causal softmax(q k^T / 8) @ v;
  concat heads; out = attn_out @ W_o.T.

Sharding (8 cores): core c handles batch b = c // 4 and head group g = c % 4
(heads 4g..4g+3). Each core computes its 4 heads' attention output and a
partial output projection against the matching 256 columns of W_o; the host
sums the 4 fp16 partials per batch (the "all-reduce" of the row-sharded W_o).

Layouts (matmul operands fp16, accumulation fp32):
  xT  [1024, 2048] = x[b].T                      e on partitions
  wqT/wkT/wvT [1024, 256] = W[256g:256g+256].T   e on partitions
  woT [256, 1024] = W_o[:, 256g:256g+256].T      d on partitions
  QT/KT [128, 2, 2048]: head pair hp, head hi at partitions 64*hi
  V     [128, 16, 4, 65]: j-chunk k, head h -> [V_h | ones] (ones col gives
        the softmax denominator for free from the same AV matmul)

Column-major attention schedule: queries are processed in 512-column blocks
t = s//512, alternating head pairs (hp0 t, hp1 t, hp0 t+1, ...). Per block:
scores^T chunks [j-tile, 512 cols] as concurrent row-tiled matmul pairs
(K=64 at partitions 0/64), exp on ACT (scale=1/8 fused) into UT_t, causal
mask mul on diagonal 128-blocks, then AV chains accumulate [V|1]^T @ U^T
into psum [65, 512] (row 64 = Z). Unnormalized rows go to outU (partitions
0-63, indexed [dk, hp, hi, s]); Z rows stay on partition 64 where
reciprocal_approx_fast runs in place (BIR verifier: compute-engine partition
starts must be 32-aligned and all inputs must share a start), then a DMA
hops 1/Z to partition 0, gpsimd partition_broadcast fans it out, and one
mul produces normalized out^T [128, 2, S] for the final projection.

QK/V projections and the final projection are emitted between score chunks
(inline or as generator "fillers" pumped once per chunk) so TensorE always
has queued work while ScalarE streams the exps - this keeps the PE busy and
HAM at the warm 2.4 GHz clock, and spreads the final projection across the
attention phase instead of a serial tail.
"""

from collections import deque

import numpy as np

from concourse import bass_utils, mybir, tile
from concourse import bacc

S = 2048
D = 1024
HPC = 4        # heads per core
DK = 64
DC = HPC * DK  # 256 d-columns per core
NCORES = 8
EC = D // 128  # 8 e-chunks
NJT = S // 128  # 16 j-tiles
NT = S // 512   # 4 query column blocks

FP16 = mybir.dt.float16
FP32 = mybir.dt.float32
BF16 = mybir.dt.bfloat16

DEBUG_DUMP = False  # set True (before _build) to DMA out intermediates


def _build():
    nc = bacc.Bacc("TRN2", target_bir_lowering=False, debug=False)

    xT_d = nc.dram_tensor("xT", [D, S], FP16, kind="ExternalInput")
    wq_d = nc.dram_tensor("wqT", [D, DC], FP16, kind="ExternalInput")
    wk_d = nc.dram_tensor("wkT", [D, DC], FP16, kind="ExternalInput")
    wv_d = nc.dram_tensor("wvT", [D, DC], FP16, kind="ExternalInput")
    wo_d = nc.dram_tensor("woT", [DC, D], FP16, kind="ExternalInput")
    mask_d = nc.dram_tensor("mask", [128, 2, 128], FP16, kind="ExternalInput")
    out_d = nc.dram_tensor("out", [S, D], FP16, kind="ExternalOutput")
    dbg = {}
    if DEBUG_DUMP:
        dbg["QT"] = nc.dram_tensor("dbg_QT", [128, 2, S], FP16, kind="ExternalOutput")
        dbg["KT"] = nc.dram_tensor("dbg_KT", [128, 2, S], FP16, kind="ExternalOutput")
        dbg["V"] = nc.dram_tensor(
            "dbg_V", [128, NJT, HPC, 65], FP16, kind="ExternalOutput"
        )
        dbg["outU"] = nc.dram_tensor(
            "dbg_outU", [64, 2, 2, S], FP32, kind="ExternalOutput"
        )
        dbg["outN"] = nc.dram_tensor(
            "dbg_outN", [128, 2, S], FP16, kind="ExternalOutput"
        )

    with tile.TileContext(nc) as tc:
        with (
            tc.tile_pool(name="const", bufs=1) as const,
            tc.tile_pool(name="work", bufs=1) as work,
            tc.tile_pool(name="ut", bufs=1) as utp,
            tc.tile_pool(name="outs", bufs=3) as outs,
            tc.tile_pool(name="norm", bufs=2) as normp,
            tc.tile_pool(name="norm1", bufs=1) as normp1,
            tc.tile_pool(name="psS", bufs=2, space="PSUM") as psS,
            tc.tile_pool(name="psA", bufs=2, space="PSUM") as psA,
            tc.tile_pool(name="psV", bufs=2, space="PSUM") as psV,
        ):
            # ---- warm the exp table while DMAs run ----
            scr = const.tile([1, 16], FP32)
            scr2 = const.tile([1, 16], FP32)
            nc.vector.memset(scr, 0.0)
            nc.scalar.activation(
                out=scr2, in_=scr, func=mybir.ActivationFunctionType.Exp
            )

            # ---- load inputs (xT st-block-major so QK proj can start early) ----
            wq = const.tile([128, EC, DC], FP16)
            wk = const.tile([128, EC, DC], FP16)
            wv = const.tile([128, EC, DC], FP16)
            xT = const.tile([128, EC, S], FP16)
            nc.sync.dma_start(out=wq, in_=wq_d.rearrange("(c p) d -> p c d", p=128))
            nc.sync.dma_start(out=wk, in_=wk_d.rearrange("(c p) d -> p c d", p=128))
            for c in range(EC):
                nc.sync.dma_start(
                    out=xT[:, c, 0:512], in_=xT_d[128 * c : 128 * (c + 1), 0:512]
                )
            nc.sync.dma_start(out=wv, in_=wv_d.rearrange("(c p) d -> p c d", p=128))
            for st in range(1, 4):
                for c in range(EC):
                    nc.sync.dma_start(
                        out=xT[:, c, 512 * st : 512 * (st + 1)],
                        in_=xT_d[128 * c : 128 * (c + 1), 512 * st : 512 * (st + 1)],
                    )
            wo = const.tile([128, 2, D], FP16)
            nc.sync.dma_start(out=wo, in_=wo_d.rearrange("(c p) d -> p c d", p=128))
            mask = const.tile([128, 2, 128], FP16)
            nc.sync.dma_start(out=mask, in_=mask_d[:, :, :])

            QT = work.tile([128, 2, S], FP16)
            KT = work.tile([128, 2, S], FP16)
            V = work.tile([128, NJT, HPC, 65], FP16)
            outU = work.tile([64, 2, 2, S], FP32)  # [dk, hp, hi, s] unnormalized
            outN = work.tile([128, 2, S], FP16)    # normalized out^T

            nc.vector.memset(V[:, :, :, 64:65], 1.0)

            # ---- generator-based TensorE fillers -------------------------
            filler = deque()

            def pump(n=1):
                for _ in range(n):
                    while filler:
                        try:
                            next(filler[0])
                            break
                        except StopIteration:
                            filler.popleft()
                    else:
                        break

            def drain_fillers():
                while filler:
                    try:
                        next(filler[0])
                    except StopIteration:
                        filler.popleft()

            def emit_qk(hp, st):
                for w_t, dst in ((wq, QT), (wk, KT)):
                    ps = psV.tile([128, 512], FP32, tag="proj")
                    for c in range(EC):
                        nc.tensor.matmul(
                            ps,
                            w_t[:, c, 128 * hp : 128 * (hp + 1)],
                            xT[:, c, 512 * st : 512 * (st + 1)],
                            start=(c == 0),
                            stop=(c == EC - 1),
                        )
                    nc.scalar.copy(
                        out=dst[:, hp, 512 * st : 512 * (st + 1)], in_=ps
                    )

            def emit_v(jt):
                ps = psV.tile([128, 512], FP32, tag="proj")
                psd = ps[:, 0:DC]
                for c in range(EC):
                    nc.tensor.matmul(
                        psd,
                        xT[:, c, 128 * jt : 128 * (jt + 1)],
                        wv[:, c, :],
                        start=(c == 0),
                        stop=(c == EC - 1),
                    )
                nc.scalar.copy(
                    out=V[:, jt, :, 0:64],
                    in_=psd.rearrange("p (h d) -> p h d", h=HPC),
                )

            def av_chain(hp, hi, t, UT_t, Zs):
                h = 2 * hp + hi
                psa = psA.tile([65, 512], FP32, tag="av")
                kmax = 4 * t + 4
                for k in range(kmax):
                    off = max(0, 128 * k - 512 * t)
                    n = 512 - off
                    nc.tensor.matmul(
                        psa[:, off : off + n],
                        V[:, k, h, :],
                        UT_t[:, hi, k, off : off + n],
                        start=(k == 0),
                        stop=(k == kmax - 1),
                    )
                    if (k + 1) % 4 == 0 and k + 1 < kmax:
                        yield
                nc.vector.tensor_copy(
                    out=outU[:, hp, hi, 512 * t : 512 * (t + 1)],
                    in_=psa[0:64, :],
                )
                nc.vector.tensor_copy(out=Zs[64:65, hi, :], in_=psa[64:65, :])

            def norm_block(hp, t, Zs):
                # DMA both heads' Z rows to partition 0 (compute engines
                # mis-address non-zero partition offsets on 1-row operands),
                # then one reciprocal + per-head broadcast from free-dim slices
                zr = normp1.tile([1, 2, 512], FP32, tag="zrow")
                nc.gpsimd.dma_start(out=zr, in_=Zs[64:65, :, :])
                zri = normp1.tile([1, 2, 512], FP32, tag="zri")
                nc.vector.reciprocal_approx_fast(out=zri, in_=zr)
                for hi in range(2):
                    zb = normp.tile([64, 512], FP32, tag="zb")
                    nc.gpsimd.partition_broadcast(zb, zri[0:1, hi, :])
                    nc.vector.tensor_mul(
                        outN[64 * hi : 64 * hi + 64, hp, 512 * t : 512 * (t + 1)],
                        outU[:, hp, hi, 512 * t : 512 * (t + 1)],
                        zb,
                    )
                return
                yield  # pragma: no cover (make this a generator)

            def final_proj(st):
                ob = outs.tile([128, D], FP16, tag="ob")
                for mt in range(2):
                    psf = psV.tile([128, 512], FP32, tag="proj")
                    for hp in range(2):
                        nc.tensor.matmul(
                            psf,
                            outN[:, hp, 128 * st : 128 * (st + 1)],
                            wo[:, hp, 512 * mt : 512 * (mt + 1)],
                            start=(hp == 0),
                            stop=(hp == 1),
                        )
                    nc.vector.tensor_copy(
                        out=ob[:, 512 * mt : 512 * (mt + 1)], in_=psf
                    )
                    yield
                nc.sync.dma_start(out=out_d[128 * st : 128 * (st + 1), :], in_=ob)

            def colblock(hp, t):
                UT_t = utp.tile([128, 2, 4 * t + 4, 512], FP16, tag=f"ut{hp}")
                Zs = normp.tile([65, 2, 512], FP32, tag="zs")
                for jt in range(4 * t + 4):
                    off = max(0, 128 * jt - 512 * t)
                    cn = 512 - off
                    ps = psS.tile([128, 2, 512], FP32, tag="score")
                    for hi in range(2):
                        ho = 64 * hi
                        nc.tensor.matmul(
                            ps[:, hi, 0:cn],
                            KT[ho : ho + 64, hp, 128 * jt : 128 * (jt + 1)],
                            QT[ho : ho + 64, hp, 512 * t + off : 512 * (t + 1)],
                            start=True,
                            stop=True,
                        )
                    nc.scalar.activation(
                        out=UT_t[:, :, jt, off : off + cn],
                        in_=ps[:, :, 0:cn],
                        func=mybir.ActivationFunctionType.Exp,
                        scale=0.125,
                    )
                    if jt >= 4 * t:
                        # diagonal 128-block: causal {0,1} mask, both heads
                        nc.vector.tensor_mul(
                            UT_t[:, :, jt, off : off + 128],
                            UT_t[:, :, jt, off : off + 128],
                            mask,
                        )
                    pump(1)
                filler.append(av_chain(hp, 0, t, UT_t, Zs))
                filler.append(av_chain(hp, 1, t, UT_t, Zs))
                filler.append(norm_block(hp, t, Zs))

            # ---- schedule ------------------------------------------------
            emit_qk(0, 0)
            colblock(0, 0)
            for jt in range(4):
                emit_v(jt)
            emit_qk(1, 0)
            colblock(1, 0)
            for st in range(4):
                filler.append(final_proj(st))
            for t in range(1, NT):
                emit_qk(0, t)
                for jt in range(4 * t, 4 * t + 4):
                    emit_v(jt)
                colblock(0, t)
                emit_qk(1, t)
                colblock(1, t)
                for st in range(4 * t, 4 * t + 4):
                    filler.append(final_proj(st))
            drain_fillers()

            if DEBUG_DUMP:
                nc.sync.dma_start(out=dbg["QT"][:, :, :], in_=QT)
                nc.sync.dma_start(out=dbg["KT"][:, :, :], in_=KT)
                nc.sync.dma_start(out=dbg["V"][:, :, :, :], in_=V)
                nc.sync.dma_start(out=dbg["outU"][:, :, :, :], in_=outU)
                nc.sync.dma_start(out=dbg["outN"][:, :, :], in_=outN)

    nc.compile()
    return nc


_NC = None


def _prep_in_maps(x, W_q, W_k, W_v, W_o):
    x = np.asarray(x, dtype=np.float32)
    W_q = np.asarray(W_q, dtype=np.float32)
    W_k = np.asarray(W_k, dtype=np.float32)
    W_v = np.asarray(W_v, dtype=np.float32)
    W_o = np.asarray(W_o, dtype=np.float32)
    mask01 = np.triu(np.ones((128, 128), dtype=np.float16))
    mask2 = np.ascontiguousarray(np.stack([mask01, mask01], axis=1))
    in_maps = []
    for c in range(NCORES):
        b, g = divmod(c, 4)
        cols = slice(DC * g, DC * (g + 1))
        in_maps.append(
            {
                "xT": np.ascontiguousarray(x[b].T).astype(np.float16),
                "wqT": np.ascontiguousarray(W_q[cols, :].T).astype(np.float16),
                "wkT": np.ascontiguousarray(W_k[cols, :].T).astype(np.float16),
                "wvT": np.ascontiguousarray(W_v[cols, :].T).astype(np.float16),
                "woT": np.ascontiguousarray(W_o[:, cols].T).astype(np.float16),
                "mask": mask2,
            }
        )
    return in_maps


def _run(x, W_q, W_k, W_v, W_o, **spmd_kwargs):
    global _NC
    if _NC is None:
        _NC = _build()
    in_maps = _prep_in_maps(x, W_q, W_k, W_v, W_o)
    res = bass_utils.run_bass_kernel_spmd(
        _NC, in_maps, core_ids=list(range(NCORES)), **spmd_kwargs
    )
    parts = [res.results[c]["out"].astype(np.float32) for c in range(NCORES)]
    out = np.empty((2, S, D), dtype=np.float32)
    for b in range(2):
        out[b] = parts[4 * b] + parts[4 * b + 1] + parts[4 * b + 2] + parts[4 * b + 3]
    return out, res


def kernel(x, W_q, W_k, W_v, W_o):
    out, _ = _run(x, W_q, W_k, W_v, W_o)
    return out
